# revision 1
# baseline (speedup 1.0000x reference)
"""NTM/DNC-style memory-augmented LSTM (B=128, T=1024) as a single-core
Trainium2 Bass/Tile kernel.

Strategy: the T=1024 recurrence is strictly sequential and each step takes
only a few microseconds, so any cross-core exchange (8-core AllReduce floor
~10us) costs more than it saves. Everything therefore runs on core 0 with the
batch (B=128) on the SBUF partition axis:
  - z = bias + x@W_ih.T + h@W_hh.T accumulated in PSUM by one PE matmul group
    per 512-wide bank (bias via a K=1 ones-matmul, x/h sides via PE-transposed
    lhsT tiles, weights pre-rounded to float32r for the 1-cycle/row PE path).
  - gates via ScalarE tanh only (sigmoid(x) = 0.5*tanh(x/2)+0.5) so a single
    activation table set is used (no 2.7us table swaps); softmax exp is in the
    same set.
  - l2norms via DVE Newton rsqrt (fast-inverse-sqrt seed + 2 iterations),
    sum-squares clamped at 1e-24 to reproduce the reference max(norm, 1e-12).
  - argmin(w_u) via DVE max/max_index on the negated (unnormalized) usage
    vector; first-index tie-breaking matches jnp.argmin including the t=0
    all-zero case.
  - w_r and w_u are kept unnormalized (exp-sum and rsqrt factors applied
    lazily) to shorten the per-step critical path.
"""
import sys
import numpy as np
from contextlib import ExitStack

sys.path.insert(0, '/opt/trn_rl_repo')
import concourse.bacc as bacc
import concourse.bass as bass
import concourse.tile as tile
from concourse import mybir, bass_utils

F32 = mybir.dt.float32
F32R = mybir.dt.float32r
I32 = mybir.dt.int32
U32 = mybir.dt.uint32
AF = mybir.ActivationFunctionType
ALU = mybir.AluOpType
AX = mybir.AxisListType

B, T, IN, HID, MEM = 128, 1024, 256, 256, 128
H4 = 4 * HID
GATE = float(1.0 / (1.0 + np.exp(0.4)))   # sigmoid(-0.4)
GAMMA = 0.3
MAGIC = 0x5F3759DF
U_UNROLL = 8

_CACHE = {}


def _emit_rsqrt(nc, pool, src, k, tag):
    nc.vector.tensor_scalar(src, src, 1e-24, None, ALU.max)
    ib = pool.tile([128, k], I32, tag=tag + "_i")
    nc.vector.tensor_scalar(ib, src.bitcast(I32), 1, None, ALU.logical_shift_right)
    nc.vector.tensor_scalar(ib, ib, -1, MAGIC, ALU.mult, ALU.add)
    y = ib.bitcast(F32)
    sh = pool.tile([128, k], F32, tag=tag + "_sh")
    nc.vector.tensor_scalar(sh, src, 0.5, None, ALU.mult)
    t = pool.tile([128, k], F32, tag=tag + "_t")
    for _ in range(2):
        nc.vector.tensor_tensor(t, y, y, ALU.mult)
        nc.vector.tensor_tensor(t, t, sh, ALU.mult)
        nc.vector.tensor_scalar(t, t, -1.0, 1.5, ALU.mult, ALU.add)
        nc.vector.tensor_tensor(y, y, t, ALU.mult)
    return y


def _build(T_run=T, U=U_UNROLL):
    nc = bacc.Bacc("TRN2", target_bir_lowering=False, debug=False)
    X = nc.dram_tensor("X", [B, T_run, IN], F32, kind="ExternalInput").ap()
    WIHT = nc.dram_tensor("WIHT", [IN, H4], F32, kind="ExternalInput").ap()
    WHHT = nc.dram_tensor("WHHT", [HID, H4], F32, kind="ExternalInput").ap()
    BIAS = nc.dram_tensor("BIAS", [1, H4], F32, kind="ExternalInput").ap()
    IOTA = nc.dram_tensor("IOTA", [128, MEM], F32, kind="ExternalInput").ap()
    IDENT = nc.dram_tensor("IDENT", [128, 128], F32, kind="ExternalInput").ap()
    OUT = nc.dram_tensor("OUT", [B, T_run, 2 * HID], F32, kind="ExternalOutput").ap()
    nchunk = T_run // U

    with tile.TileContext(nc) as tc, ExitStack() as ctx:
        const = ctx.enter_context(tc.tile_pool(name="const", bufs=1))
        state = ctx.enter_context(tc.tile_pool(name="state", bufs=1))
        xp = ctx.enter_context(tc.tile_pool(name="xp", bufs=2))
        op = ctx.enter_context(tc.tile_pool(name="op", bufs=2))
        wk = ctx.enter_context(tc.tile_pool(name="wk", bufs=2))
        psz = ctx.enter_context(tc.tile_pool(name="psz", bufs=1, space="PSUM"))
        pst = ctx.enter_context(tc.tile_pool(name="pst", bufs=2, space="PSUM"))
        psm = ctx.enter_context(tc.tile_pool(name="psm", bufs=1, space="PSUM"))

        wih = const.tile([128, 2, H4], F32)
        nc.sync.dma_start(wih[:, 0, :], WIHT[0:128, :])
        nc.sync.dma_start(wih[:, 1, :], WIHT[128:256, :])
        whh = const.tile([128, 2, H4], F32)
        nc.sync.dma_start(whh[:, 0, :], WHHT[0:128, :])
        nc.sync.dma_start(whh[:, 1, :], WHHT[128:256, :])
        biasr = const.tile([1, H4], F32)
        nc.sync.dma_start(biasr, BIAS)
        iota = const.tile([128, MEM], F32)
        nc.sync.dma_start(iota, IOTA)
        ident = const.tile([128, 128], F32)
        nc.sync.dma_start(ident, IDENT)
        ones1f = const.tile([1, 128], F32)
        nc.vector.memset(ones1f, 1.0)
        ones1 = const.tile([1, 128], F32R)
        nc.vector.tensor_copy(out=ones1, in_=ones1f)
        wihr = const.tile([128, 2, H4], F32R)
        nc.vector.tensor_copy(out=wihr, in_=wih)
        whhr = const.tile([128, 2, H4], F32R)
        nc.vector.tensor_copy(out=whhr, in_=whh)
        biasrr = const.tile([1, H4], F32R)
        nc.vector.tensor_copy(out=biasrr, in_=biasr)

        hT = state.tile([128, 2, 128], F32R)
        c = state.tile([128, HID], F32)
        Mpp = state.tile([128, 2, HID], F32)
        MT = state.tile([128, 2, MEM], F32)
        e_s = state.tile([128, MEM], F32)
        rse = state.tile([128, 1], F32)
        uP = state.tile([128, MEM], F32)
        ru = state.tile([128, 1], F32)
        for tl in (c, Mpp, MT, e_s, rse, uP, ru):
            nc.vector.memset(tl, 0.0)
        nc.vector.tensor_copy(out=hT, in_=Mpp[:, 0, :])

        def step(x_ap, o_ap, u):
            Mold = Mpp[:, u % 2, :]
            Mnew = Mpp[:, (u + 1) % 2, :]
            negu = wk.tile([128, MEM], F32, tag="negu")
            nc.vector.tensor_scalar(negu, uP, -1.0, None, ALU.mult)
            m8 = wk.tile([128, 8], F32, tag="m8")
            nc.vector.max(m8, negu)
            i8 = wk.tile([128, 8], U32, tag="i8")
            nc.vector.max_index(i8, m8, negu)
            idxf = wk.tile([128, 1], F32, tag="idxf")
            nc.vector.tensor_copy(out=idxf, in_=i8[:, 0:1])
            onehot = wk.tile([128, MEM], F32, tag="onehot")
            nc.vector.tensor_scalar(onehot, iota, idxf, None, ALU.is_equal)
            grs = wk.tile([128, 1], F32, tag="grs")
            nc.vector.tensor_scalar(grs, rse, GATE, None, ALU.mult)
            gwr = wk.tile([128, MEM], F32, tag="gwr")
            nc.vector.tensor_scalar(gwr, e_s, grs, None, ALU.mult)
            w_w = wk.tile([128, MEM], F32, tag="w_w")
            nc.vector.scalar_tensor_tensor(w_w, onehot, 1.0 - GATE, gwr, ALU.mult, ALU.add)
            gru = wk.tile([128, 1], F32, tag="gru")
            nc.vector.tensor_scalar(gru, ru, GAMMA, None, ALU.mult)
            nc.vector.scalar_tensor_tensor(uP, uP, gru, w_w, ALU.mult, ALU.add)

            xT = wk.tile([128, 2, 128], F32R, tag="xT")
            for k in range(2):
                tp = pst.tile([128, 128], F32, tag="tp")
                nc.tensor.transpose(tp, x_ap[:, k * 128:(k + 1) * 128], ident)
                nc.scalar.copy(xT[:, k, :], tp)

            zb = []
            for b_i in range(2):
                z = psz.tile([128, 512], F32, tag=f"z{b_i}")
                sl = slice(b_i * 512, (b_i + 1) * 512)
                nc.tensor.matmul(z, ones1, biasrr[:, sl], start=True, stop=False)
                nc.tensor.matmul(z, xT[:, 0, :], wihr[:, 0, sl], start=False, stop=False)
                nc.tensor.matmul(z, xT[:, 1, :], wihr[:, 1, sl], start=False, stop=False)
                nc.tensor.matmul(z, hT[:, 0, :], whhr[:, 0, sl], start=False, stop=False)
                nc.tensor.matmul(z, hT[:, 1, :], whhr[:, 1, sl], start=False, stop=True)
                zb.append(z)
            z0, z1 = zb  # z0=[i,f], z1=[g,o]

            thif = wk.tile([128, 512], F32, tag="thif")
            nc.scalar.activation(thif, z0, AF.Tanh, scale=0.5)
            sif = wk.tile([128, 512], F32, tag="sif")
            nc.vector.tensor_scalar(sif, thif, 0.5, 0.5, ALU.mult, ALU.add)
            tg = wk.tile([128, 256], F32, tag="tg")
            nc.scalar.activation(tg, z1[:, 0:256], AF.Tanh)
            tho = wk.tile([128, 256], F32, tag="tho")
            nc.scalar.activation(tho, z1[:, 256:512], AF.Tanh, scale=0.5)
            so = wk.tile([128, 256], F32, tag="so")
            nc.vector.tensor_scalar(so, tho, 0.5, 0.5, ALU.mult, ALU.add)

            t1 = wk.tile([128, 256], F32, tag="t1")
            nc.vector.tensor_tensor(t1, sif[:, 256:512], c, ALU.mult)
            t2 = wk.tile([128, 256], F32, tag="t2")
            nc.vector.tensor_tensor(t2, sif[:, 0:256], tg, ALU.mult)
            nc.vector.tensor_tensor(c, t1, t2, ALU.add)
            tcn = wk.tile([128, 256], F32, tag="tcn")
            nc.scalar.activation(tcn, c, AF.Tanh)
            h = o_ap[:, 0:256]
            nc.vector.tensor_tensor(h, so, tcn, ALU.mult)

            nrm = wk.tile([128, 2], F32, tag="nrm")
            sq = wk.tile([128, 256], F32, tag="sq")
            nc.vector.scalar_tensor_tensor(sq, h, 1.0, h, ALU.mult, ALU.mult,
                                           accum_out=nrm[:, 1:2])

            for k in range(2):
                tp = pst.tile([128, 128], F32, tag="tp")
                nc.tensor.transpose(tp, h[:, k * 128:(k + 1) * 128], ident)
                nc.vector.tensor_copy(out=hT[:, k, :], in_=tp)

            dps = psm.tile([128, 256], F32, tag="dps")
            nc.tensor.matmul(dps, w_w, h, start=True, stop=True)
            MpD = wk.tile([128, 256], F32, tag="MpD")
            nc.vector.tensor_tensor(MpD, dps, Mold, ALU.add)
            sqm = wk.tile([128, 256], F32, tag="sqm")
            nc.vector.scalar_tensor_tensor(sqm, MpD, 1.0, MpD, ALU.mult, ALU.mult,
                                           accum_out=nrm[:, 0:1])
            rs = _emit_rsqrt(nc, wk, nrm, 2, "rsA")
            nc.vector.tensor_scalar(Mnew, MpD, rs[:, 0:1], None, ALU.mult)
            for k in range(2):
                tp = pst.tile([128, 128], F32, tag="tp")
                nc.tensor.transpose(tp, Mnew[:, k * 128:(k + 1) * 128], ident)
                nc.vector.tensor_copy(out=MT[:, k, :], in_=tp)

            ips = psm.tile([128, MEM], F32, tag="ips")
            nc.tensor.matmul(ips, hT[:, 0, :].bitcast(F32), MT[:, 0, :], start=True, stop=False)
            nc.tensor.matmul(ips, hT[:, 1, :].bitcast(F32), MT[:, 1, :], start=False, stop=True)
            sc = wk.tile([128, MEM], F32, tag="sc")
            nc.vector.tensor_scalar(sc, ips, rs[:, 1:2], None, ALU.mult)
            mx = wk.tile([128, 1], F32, tag="mx")
            nc.vector.tensor_reduce(mx, sc, AX.X, ALU.max)
            bm = wk.tile([128, 1], F32, tag="bm")
            nc.vector.tensor_scalar(bm, mx, -1.0, None, ALU.mult)
            se = wk.tile([128, 1], F32, tag="se")
            nc.scalar.activation(e_s, sc, AF.Exp, bias=bm, scale=1.0, accum_out=se)
            nc.vector.reciprocal(rse, se)

            eT = wk.tile([128, MEM], F32, tag="eT")
            tp = pst.tile([128, 128], F32, tag="tp")
            nc.tensor.transpose(tp, e_s, ident)
            nc.vector.tensor_copy(out=eT, in_=tp)
            rps = psm.tile([128, 256], F32, tag="rps")
            nc.tensor.matmul(rps, eT, Mold, start=True, stop=True)
            nc.vector.tensor_scalar(o_ap[:, 256:512], rps, rse, None, ALU.mult)

            nc.vector.scalar_tensor_tensor(uP, e_s, rse, uP, ALU.mult, ALU.add)
            nrb = wk.tile([128, 1], F32, tag="nrb")
            squ = wk.tile([128, MEM], F32, tag="squ")
            nc.vector.scalar_tensor_tensor(squ, uP, 1.0, uP, ALU.mult, ALU.mult,
                                           accum_out=nrb)
            rb = _emit_rsqrt(nc, wk, nrb, 1, "rsB")
            nc.vector.tensor_copy(out=ru, in_=rb)

        if nchunk > 1:
            with tc.For_i(0, nchunk, 1, staggered_reset=True,
                          hint_engines=(mybir.EngineType.DVE,
                                        mybir.EngineType.PE,
                                        mybir.EngineType.Activation)) as ic:
                xt = xp.tile([128, U, IN], F32)
                nc.sync.dma_start(xt, X[:, bass.ts(ic, U), :])
                ot = op.tile([128, U, 2 * HID], F32)
                for u in range(U):
                    step(xt[:, u, :], ot[:, u, :], u)
                nc.sync.dma_start(OUT[:, bass.ts(ic, U), :], ot)
        else:
            xt = xp.tile([128, U, IN], F32)
            nc.sync.dma_start(xt, X[:, 0:U, :])
            ot = op.tile([128, U, 2 * HID], F32)
            for u in range(U):
                step(xt[:, u, :], ot[:, u, :], u)
            nc.sync.dma_start(OUT[:, 0:U, :], ot)

    nc.compile()
    return nc


def kernel(X, W_ih, W_hh, b_ih, b_hh):
    X = np.ascontiguousarray(np.asarray(X, dtype=np.float32))
    in_map = {
        "X": X,
        "WIHT": np.ascontiguousarray(np.asarray(W_ih, np.float32).T),
        "WHHT": np.ascontiguousarray(np.asarray(W_hh, np.float32).T),
        "BIAS": np.ascontiguousarray(
            (np.asarray(b_ih, np.float32) + np.asarray(b_hh, np.float32)).reshape(1, H4)),
        "IOTA": np.tile(np.arange(MEM, dtype=np.float32), (128, 1)),
        "IDENT": np.eye(128, dtype=np.float32),
    }
    if "nc" not in _CACHE:
        _CACHE["nc"] = _build(T, U_UNROLL)
    res = bass_utils.run_bass_kernel_spmd(_CACHE["nc"], [in_map], core_ids=[0])
    return res.results[0]["OUT"]



# revision 4
# speedup vs baseline: 3.1425x; 3.1425x over previous
"""NTM/DNC-style memory-augmented LSTM (B=128, T=1024) on one TRN2 core,
tuned for the axon tunnel: wall time is transfer-dominated (~40 MB/s), so
X ships as int16 (67 MB, scale folded into W_ih host-side), OUT returns as
int16 (134 MB, fixed scale 32766 — |out| < 1 analytically), and the run is
segmented over T so H2D / execute / D2H pipeline.

Kernel structure (per step, reference order):
  - z = bias + x@W_ih.T + h@W_hh.T accumulated in PSUM by full-fp32 PE
    matmuls (bias via a K=1 ones-matmul, x/h sides via PE-transposed lhsT).
  - gates via ScalarE tanh only (sigmoid(x) = 0.5*tanh(x/2)+0.5); softmax
    exp shares the same activation-table set.
  - w_r softmax against the PRE-update M (matches reference ordering);
    the M update runs off the read critical path.
  - l2norms via DVE Newton rsqrt (magic seed + 2 iters, clamp 1e-24);
    argmin via DVE max/max_index on -uP (first-index tie-break).
  - recurrent state (hT, c, MT, e_s, uP, rse, ru, M) packed in one
    [128, 1282] fp32 DRAM tensor so segments chain on-device.
"""
import sys
import numpy as np
from contextlib import ExitStack

sys.path.insert(0, '/opt/trn_rl_repo')
import concourse.bacc as bacc
import concourse.bass as bass
import concourse.tile as tile
from concourse import mybir

F32 = mybir.dt.float32
I16 = mybir.dt.int16
I32 = mybir.dt.int32
U32 = mybir.dt.uint32
AF = mybir.ActivationFunctionType
ALU = mybir.AluOpType
AX = mybir.AxisListType

B, T, IN, HID, MEM = 128, 1024, 256, 256, 128
H4 = 4 * HID
GATE = float(1.0 / (1.0 + np.exp(0.4)))   # sigmoid(-0.4)
GAMMA = 0.3
MAGIC = 0x5F3759DF
U_UNROLL = 8
T_SEG = 128
N_SEG = T // T_SEG
OSCALE = 32766.0

# packed state layout (fp32 columns per partition)
S_HT, S_C, S_MT, S_ES, S_UP, S_RSE, S_RU, S_M = (
    0, 256, 512, 768, 896, 1024, 1025, 1026)
SW = 1282

_CACHE = {}


def _emit_rsqrt(nc, pool, src, k, tag):
    """rsqrt(max(src, 1e-24)) via fast-inverse-sqrt seed + 2 Newton iters."""
    nc.vector.tensor_scalar(src, src, 1e-24, None, ALU.max)
    ib = pool.tile([128, k], I32, tag=tag + "_i")
    nc.vector.tensor_scalar(ib, src.bitcast(I32), 1, None, ALU.logical_shift_right)
    nc.vector.tensor_scalar(ib, ib, -1, MAGIC, ALU.mult, ALU.add)
    y = ib.bitcast(F32)
    sh = pool.tile([128, k], F32, tag=tag + "_sh")
    nc.vector.tensor_scalar(sh, src, 0.5, None, ALU.mult)
    t = pool.tile([128, k], F32, tag=tag + "_t")
    for _ in range(2):
        nc.vector.tensor_tensor(t, y, y, ALU.mult)
        nc.vector.tensor_tensor(t, t, sh, ALU.mult)
        nc.vector.tensor_scalar(t, t, -1.0, 1.5, ALU.mult, ALU.add)
        nc.vector.tensor_tensor(y, y, t, ALU.mult)
    return y


def _build(T_run, U=U_UNROLL):
    nc = bacc.Bacc("TRN2", target_bir_lowering=False, debug=False)
    XQ = nc.dram_tensor("XQ", [B, T_run, IN], I16, kind="ExternalInput").ap()
    WIHT = nc.dram_tensor("WIHT", [IN, H4], F32, kind="ExternalInput").ap()
    WHHT = nc.dram_tensor("WHHT", [HID, H4], F32, kind="ExternalInput").ap()
    BIAS = nc.dram_tensor("BIAS", [1, H4], F32, kind="ExternalInput").ap()
    IOTA = nc.dram_tensor("IOTA", [128, MEM], F32, kind="ExternalInput").ap()
    IDENT = nc.dram_tensor("IDENT", [128, 128], F32, kind="ExternalInput").ap()
    SIN = nc.dram_tensor("SIN", [128, SW], F32, kind="ExternalInput").ap()
    OUT = nc.dram_tensor("OUT", [B, T_run, 2 * HID], I16, kind="ExternalOutput").ap()
    SOUT = nc.dram_tensor("SOUT", [128, SW], F32, kind="ExternalOutput").ap()
    nchunk = T_run // U

    with tile.TileContext(nc) as tc, ExitStack() as ctx:
        const = ctx.enter_context(tc.tile_pool(name="const", bufs=1))
        state = ctx.enter_context(tc.tile_pool(name="state", bufs=1))
        xp = ctx.enter_context(tc.tile_pool(name="xp", bufs=2))
        xf = ctx.enter_context(tc.tile_pool(name="xf", bufs=2))
        op = ctx.enter_context(tc.tile_pool(name="op", bufs=2))
        wk = ctx.enter_context(tc.tile_pool(name="wk", bufs=2))
        psz = ctx.enter_context(tc.tile_pool(name="psz", bufs=1, space="PSUM"))
        pst = ctx.enter_context(tc.tile_pool(name="pst", bufs=2, space="PSUM"))
        psm = ctx.enter_context(tc.tile_pool(name="psm", bufs=1, space="PSUM"))

        wih = const.tile([128, 2, H4], F32)
        nc.sync.dma_start(wih[:, 0, :], WIHT[0:128, :])
        nc.sync.dma_start(wih[:, 1, :], WIHT[128:256, :])
        whh = const.tile([128, 2, H4], F32)
        nc.sync.dma_start(whh[:, 0, :], WHHT[0:128, :])
        nc.sync.dma_start(whh[:, 1, :], WHHT[128:256, :])
        biasr = const.tile([1, H4], F32)
        nc.sync.dma_start(biasr, BIAS)
        iota = const.tile([128, MEM], F32)
        nc.sync.dma_start(iota, IOTA)
        ident = const.tile([128, 128], F32)
        nc.sync.dma_start(ident, IDENT)
        ones1 = const.tile([1, 128], F32)
        nc.vector.memset(ones1, 1.0)

        st = state.tile([128, SW], F32)
        nc.sync.dma_start(st, SIN)
        Mpp = state.tile([128, 2, HID], F32)
        nc.vector.tensor_copy(out=Mpp[:, 0, :], in_=st[:, S_M:S_M + HID])

        c = st[:, S_C:S_C + HID]
        e_s = st[:, S_ES:S_ES + MEM]
        uP = st[:, S_UP:S_UP + MEM]
        rse = st[:, S_RSE:S_RSE + 1]
        ru = st[:, S_RU:S_RU + 1]

        def hT(k):
            return st[:, S_HT + k * 128:S_HT + (k + 1) * 128]

        def MT(k):
            return st[:, S_MT + k * 128:S_MT + (k + 1) * 128]

        def step(x_ap, o_ap, u):
            Mold = Mpp[:, u % 2, :]
            Mnew = Mpp[:, (u + 1) % 2, :]

            # (A) write weights from previous-step state
            negu = wk.tile([128, MEM], F32, tag="negu")
            nc.vector.tensor_scalar(negu, uP, -1.0, None, ALU.mult)
            m8 = wk.tile([128, 8], F32, tag="m8")
            nc.vector.max(m8, negu)
            i8 = wk.tile([128, 8], U32, tag="i8")
            nc.vector.max_index(i8, m8, negu)
            idxf = wk.tile([128, 1], F32, tag="idxf")
            nc.vector.tensor_copy(out=idxf, in_=i8[:, 0:1])
            onehot = wk.tile([128, MEM], F32, tag="onehot")
            nc.vector.tensor_scalar(onehot, iota, idxf, None, ALU.is_equal)
            grs = wk.tile([128, 1], F32, tag="grs")
            nc.vector.tensor_scalar(grs, rse, GATE, None, ALU.mult)
            gwr = wk.tile([128, MEM], F32, tag="gwr")
            nc.vector.tensor_scalar(gwr, e_s, grs, None, ALU.mult)
            w_w = wk.tile([128, MEM], F32, tag="w_w")
            nc.vector.scalar_tensor_tensor(w_w, onehot, 1.0 - GATE, gwr, ALU.mult, ALU.add)
            gru = wk.tile([128, 1], F32, tag="gru")
            nc.vector.tensor_scalar(gru, ru, GAMMA, None, ALU.mult)
            nc.vector.scalar_tensor_tensor(uP, uP, gru, w_w, ALU.mult, ALU.add)

            # (B) LSTM cell
            xT = wk.tile([128, 2, 128], F32, tag="xT")
            for k in range(2):
                tp = pst.tile([128, 128], F32, tag="tp")
                nc.tensor.transpose(tp, x_ap[:, k * 128:(k + 1) * 128], ident)
                nc.scalar.copy(xT[:, k, :], tp)

            zb = []
            for b_i in range(2):
                z = psz.tile([128, 512], F32, tag=f"z{b_i}")
                sl = slice(b_i * 512, (b_i + 1) * 512)
                nc.tensor.matmul(z, ones1, biasr[:, sl], start=True, stop=False)
                nc.tensor.matmul(z, xT[:, 0, :], wih[:, 0, sl], start=False, stop=False)
                nc.tensor.matmul(z, xT[:, 1, :], wih[:, 1, sl], start=False, stop=False)
                nc.tensor.matmul(z, hT(0), whh[:, 0, sl], start=False, stop=False)
                nc.tensor.matmul(z, hT(1), whh[:, 1, sl], start=False, stop=True)
                zb.append(z)
            z0, z1 = zb  # z0=[i,f], z1=[g,o]

            thif = wk.tile([128, 512], F32, tag="thif")
            nc.scalar.activation(thif, z0, AF.Tanh, scale=0.5)
            sif = wk.tile([128, 512], F32, tag="sif")
            nc.vector.tensor_scalar(sif, thif, 0.5, 0.5, ALU.mult, ALU.add)
            tg = wk.tile([128, 256], F32, tag="tg")
            nc.scalar.activation(tg, z1[:, 0:256], AF.Tanh)
            tho = wk.tile([128, 256], F32, tag="tho")
            nc.scalar.activation(tho, z1[:, 256:512], AF.Tanh, scale=0.5)
            so = wk.tile([128, 256], F32, tag="so")
            nc.vector.tensor_scalar(so, tho, 0.5, 0.5, ALU.mult, ALU.add)

            t1 = wk.tile([128, 256], F32, tag="t1")
            nc.vector.tensor_tensor(t1, sif[:, 256:512], c, ALU.mult)
            t2 = wk.tile([128, 256], F32, tag="t2")
            nc.vector.tensor_tensor(t2, sif[:, 0:256], tg, ALU.mult)
            nc.vector.tensor_tensor(c, t1, t2, ALU.add)
            tcn = wk.tile([128, 256], F32, tag="tcn")
            nc.scalar.activation(tcn, c, AF.Tanh)
            h = wk.tile([128, 256], F32, tag="h")
            nc.vector.tensor_tensor(h, so, tcn, ALU.mult)
            nc.vector.tensor_scalar(o_ap[:, 0:256], h, OSCALE, None, ALU.mult)

            nh = wk.tile([128, 1], F32, tag="nh")
            sq = wk.tile([128, 256], F32, tag="sq")
            nc.vector.scalar_tensor_tensor(sq, h, 1.0, h, ALU.mult, ALU.mult,
                                           accum_out=nh)
            rh = _emit_rsqrt(nc, wk, nh, 1, "rsH")

            for k in range(2):
                tp = pst.tile([128, 128], F32, tag="tp")
                nc.tensor.transpose(tp, h[:, k * 128:(k + 1) * 128], ident)
                nc.vector.tensor_copy(out=hT(k), in_=tp)

            # (C) read head against PRE-update M (reference ordering)
            ips = psm.tile([128, MEM], F32, tag="ips")
            nc.tensor.matmul(ips, hT(0), MT(0), start=True, stop=False)
            nc.tensor.matmul(ips, hT(1), MT(1), start=False, stop=True)
            sc = wk.tile([128, MEM], F32, tag="sc")
            nc.vector.tensor_scalar(sc, ips, rh, None, ALU.mult)
            mx = wk.tile([128, 1], F32, tag="mx")
            nc.vector.tensor_reduce(mx, sc, AX.X, ALU.max)
            bm = wk.tile([128, 1], F32, tag="bm")
            nc.vector.tensor_scalar(bm, mx, -1.0, None, ALU.mult)
            se = wk.tile([128, 1], F32, tag="se")
            nc.scalar.activation(e_s, sc, AF.Exp, bias=bm, scale=1.0, accum_out=se)
            nc.vector.reciprocal(rse, se)

            eT = wk.tile([128, MEM], F32, tag="eT")
            tp = pst.tile([128, 128], F32, tag="tp")
            nc.tensor.transpose(tp, e_s, ident)
            nc.vector.tensor_copy(out=eT, in_=tp)
            rps = psm.tile([128, 256], F32, tag="rps")
            nc.tensor.matmul(rps, eT, Mold, start=True, stop=True)
            nc.vector.tensor_scalar(o_ap[:, 256:512], rps, rse, OSCALE,
                                    ALU.mult, ALU.mult)

            # (D) memory update (off the read critical path)
            dps = psm.tile([128, 256], F32, tag="dps")
            nc.tensor.matmul(dps, w_w, h, start=True, stop=True)
            MpD = wk.tile([128, 256], F32, tag="MpD")
            nc.vector.tensor_tensor(MpD, dps, Mold, ALU.add)
            nm = wk.tile([128, 1], F32, tag="nm")
            sqm = wk.tile([128, 256], F32, tag="sqm")
            nc.vector.scalar_tensor_tensor(sqm, MpD, 1.0, MpD, ALU.mult, ALU.mult,
                                           accum_out=nm)
            rm = _emit_rsqrt(nc, wk, nm, 1, "rsM")
            nc.vector.tensor_scalar(Mnew, MpD, rm, None, ALU.mult)
            for k in range(2):
                tp = pst.tile([128, 128], F32, tag="tp")
                nc.tensor.transpose(tp, Mnew[:, k * 128:(k + 1) * 128], ident)
                nc.vector.tensor_copy(out=MT(k), in_=tp)

            # (E) usage update
            nc.vector.scalar_tensor_tensor(uP, e_s, rse, uP, ALU.mult, ALU.add)
            nu = wk.tile([128, 1], F32, tag="nu")
            squ = wk.tile([128, MEM], F32, tag="squ")
            nc.vector.scalar_tensor_tensor(squ, uP, 1.0, uP, ALU.mult, ALU.mult,
                                           accum_out=nu)
            rb = _emit_rsqrt(nc, wk, nu, 1, "rsU")
            nc.vector.tensor_copy(out=ru, in_=rb)

        def chunk_body(x_dram_slice, out_dram_slice):
            xq = xp.tile([128, U, IN], I16)
            nc.sync.dma_start(xq, x_dram_slice)
            xt = xf.tile([128, U, IN], F32)
            nc.vector.tensor_copy(out=xt, in_=xq)
            ot = op.tile([128, U, 2 * HID], I16)
            for u in range(U):
                step(xt[:, u, :], ot[:, u, :], u)
            nc.sync.dma_start(out_dram_slice, ot)

        if nchunk > 1:
            with tc.For_i(0, nchunk, 1, staggered_reset=True,
                          hint_engines=(mybir.EngineType.DVE,
                                        mybir.EngineType.PE,
                                        mybir.EngineType.Activation)) as ic:
                chunk_body(XQ[:, bass.ts(ic, U), :], OUT[:, bass.ts(ic, U), :])
        else:
            chunk_body(XQ[:, 0:U, :], OUT[:, 0:U, :])

        nc.vector.tensor_copy(out=st[:, S_M:S_M + HID], in_=Mpp[:, 0, :])
        nc.sync.dma_start(SOUT, st)

    nc.compile()
    return nc


def _get_engine():
    if "eng" in _CACHE:
        return _CACHE["eng"]
    import jax
    import jax.numpy as jnp
    from concourse.bass2jax import (_bass_exec_p, install_neuronx_cc_hook,
                                    partition_id_tensor)
    install_neuronx_cc_hook()
    nc = _build(T_SEG, U_UNROLL)
    in_names, out_names, out_avals, zero_specs = [], [], [], []
    for alloc in nc.m.functions[0].allocations:
        if not isinstance(alloc, mybir.MemoryLocationSet):
            continue
        name = alloc.memorylocations[0].name
        if alloc.kind == "ExternalInput":
            if name != "partition_id":
                in_names.append(name)
        elif alloc.kind == "ExternalOutput":
            out_names.append(name)
            shape = tuple(alloc.tensor_shape)
            dtype = mybir.dt.np(alloc.dtype)
            out_avals.append(jax.core.ShapedArray(shape, dtype))
            zero_specs.append((shape, dtype))
    n_params = len(in_names)
    has_pid = nc.partition_id_tensor is not None
    all_in_names = tuple(in_names + out_names
                         + (["partition_id"] if has_pid else []))

    def _body(*args):
        operands = list(args)
        if has_pid:
            operands.append(partition_id_tensor())
        return tuple(_bass_exec_p.bind(
            *operands,
            out_avals=tuple(out_avals),
            in_names=all_in_names,
            out_names=tuple(out_names),
            lowering_input_output_aliases=(),
            sim_require_finite=True,
            sim_require_nnan=True,
            nc=nc,
        ))

    donate = tuple(range(n_params, n_params + len(out_names)))
    jitted = jax.jit(_body, donate_argnums=donate, keep_unused=True)
    mk_zeros = jax.jit(
        lambda: tuple(jnp.zeros(s, d) for s, d in zero_specs))
    mk_state0 = jax.jit(lambda: jnp.zeros((128, SW), np.float32))
    eng = {"nc": nc, "jitted": jitted, "mk_zeros": mk_zeros,
           "mk_state0": mk_state0, "in_names": in_names,
           "out_names": out_names, "jax": jax}
    _CACHE["eng"] = eng
    return eng


def kernel(X, W_ih, W_hh, b_ih, b_hh):
    eng = _get_engine()
    jax = eng["jax"]
    dev = jax.devices()[0]

    X = np.asarray(X, np.float32)
    amax = float(np.abs(X).max())
    xs = amax / 32767.0 if amax > 0 else 1.0
    XQ = np.rint(X * (1.0 / xs)).astype(np.int16)

    consts = {
        "WIHT": np.ascontiguousarray(np.asarray(W_ih, np.float32).T) * np.float32(xs),
        "WHHT": np.ascontiguousarray(np.asarray(W_hh, np.float32).T),
        "BIAS": np.ascontiguousarray(
            (np.asarray(b_ih, np.float32) + np.asarray(b_hh, np.float32)
             ).reshape(1, H4)),
        "IOTA": np.tile(np.arange(MEM, dtype=np.float32), (128, 1)),
        "IDENT": np.eye(128, dtype=np.float32),
    }
    dev_consts = {k: jax.device_put(v, dev) for k, v in consts.items()}

    # enqueue per-segment H2D of X up front (async; tunnel streams in order)
    xsegs = [jax.device_put(
        np.ascontiguousarray(XQ[:, s * T_SEG:(s + 1) * T_SEG, :]), dev)
        for s in range(N_SEG)]

    state = eng["mk_state0"]()
    outs = []
    for s in range(N_SEG):
        zeros = eng["mk_zeros"]()
        in_map = dict(dev_consts)
        in_map["XQ"] = xsegs[s]
        in_map["SIN"] = state
        args = [in_map[n] for n in eng["in_names"]]
        res = eng["jitted"](*args, *zeros)
        res_map = dict(zip(eng["out_names"], res))
        o = res_map["OUT"]
        o.copy_to_host_async()
        outs.append(o)
        state = res_map["SOUT"]

    full = np.empty((B, T, 2 * HID), np.float32)
    inv = np.float32(1.0 / OSCALE)
    for s, o in enumerate(outs):
        np.multiply(np.asarray(o), inv,
                    out=full[:, s * T_SEG:(s + 1) * T_SEG, :])
    return full


# revision 11
# speedup vs baseline: 4.5183x; 1.4378x over previous
"""NTM/DNC-style memory-augmented LSTM (B=128, T=1024) on one TRN2 core,
tuned for the axon tunnel: wall time is transfer-dominated (~40 MB/s), so
X ships as int16 (67 MB, scale folded into W_ih host-side), OUT returns as
int16 (134 MB, fixed scale 32766 — |out| < 1 analytically), and the run is
segmented over T so H2D / execute / D2H pipeline.

Kernel structure (per step, reference order):
  - z = bias + x@W_ih.T + h@W_hh.T accumulated in PSUM by full-fp32 PE
    matmuls (bias via a K=1 ones-matmul, x/h sides via PE-transposed lhsT).
  - gates via ScalarE tanh only (sigmoid(x) = 0.5*tanh(x/2)+0.5); softmax
    exp shares the same activation-table set.
  - w_r softmax against the PRE-update M (matches reference ordering);
    the M update runs off the read critical path.
  - l2norms via DVE Newton rsqrt (magic seed + 2 iters, clamp 1e-24);
    argmin via DVE max/max_index on -uP (first-index tie-break).
  - recurrent state (hT, c, MT, e_s, uP, rse, ru, M) packed in one
    [128, 1282] fp32 DRAM tensor so segments chain on-device.
"""
import sys
import numpy as np
from contextlib import ExitStack

sys.path.insert(0, '/opt/trn_rl_repo')
import concourse.bacc as bacc
import concourse.bass as bass
import concourse.tile as tile
from concourse import mybir

F32 = mybir.dt.float32
I16 = mybir.dt.int16
U8 = mybir.dt.uint8
I32 = mybir.dt.int32
U32 = mybir.dt.uint32
AF = mybir.ActivationFunctionType
ALU = mybir.AluOpType
AX = mybir.AxisListType

B, T, IN, HID, MEM = 128, 1024, 256, 256, 128
H4 = 4 * HID
GATE = float(1.0 / (1.0 + np.exp(0.4)))   # sigmoid(-0.4)
GAMMA = 0.3
MAGIC = 0x5F3759DF
U_UNROLL = 8
T_SEG = 128
N_SEG = T // T_SEG
# OUT wire format: 'u8' = uint8 q = trunc(out*127 + 127.5)  (err <= 0.5/127)
#                  'i16' = int16 q = out*32766              (err <= ~3e-5)
OUT_FMT = 'u8'
OSCALE = 32766.0
U8S = 127.0

# packed state layout (fp32 columns per partition)
S_HT, S_C, S_MT, S_ES, S_UP, S_RSE, S_RU, S_M = (
    0, 256, 512, 768, 896, 1024, 1025, 1026)
SW = 1282

_CACHE = {}


def _emit_rsqrt(nc, pool, src, k, tag):
    """rsqrt(max(src, 1e-24)) via fast-inverse-sqrt seed + 2 Newton iters."""
    nc.vector.tensor_scalar(src, src, 1e-24, None, ALU.max)
    ib = pool.tile([128, k], I32, tag=tag + "_i")
    nc.vector.tensor_scalar(ib, src.bitcast(I32), 1, None, ALU.logical_shift_right)
    nc.vector.tensor_scalar(ib, ib, -1, MAGIC, ALU.mult, ALU.add)
    y = ib.bitcast(F32)
    sh = pool.tile([128, k], F32, tag=tag + "_sh")
    nc.vector.tensor_scalar(sh, src, 0.5, None, ALU.mult)
    t = pool.tile([128, k], F32, tag=tag + "_t")
    for _ in range(2):
        nc.vector.tensor_tensor(t, y, y, ALU.mult)
        nc.vector.tensor_tensor(t, t, sh, ALU.mult)
        nc.vector.tensor_scalar(t, t, -1.0, 1.5, ALU.mult, ALU.add)
        nc.vector.tensor_tensor(y, y, t, ALU.mult)
    return y


def _build(T_run, U=U_UNROLL):
    nc = bacc.Bacc("TRN2", target_bir_lowering=False, debug=False)
    XQ = nc.dram_tensor("XQ", [B, T_run, IN], I16, kind="ExternalInput").ap()
    WIHT = nc.dram_tensor("WIHT", [IN, H4], F32, kind="ExternalInput").ap()
    WHHT = nc.dram_tensor("WHHT", [HID, H4], F32, kind="ExternalInput").ap()
    BIAS = nc.dram_tensor("BIAS", [1, H4], F32, kind="ExternalInput").ap()
    IOTA = nc.dram_tensor("IOTA", [128, MEM], F32, kind="ExternalInput").ap()
    IDENT = nc.dram_tensor("IDENT", [128, 128], F32, kind="ExternalInput").ap()
    SIN = nc.dram_tensor("SIN", [128, SW], F32, kind="ExternalInput").ap()
    ODT = U8 if OUT_FMT == 'u8' else I16
    OUT = nc.dram_tensor("OUT", [B, T_run, 2 * HID], ODT, kind="ExternalOutput").ap()
    SOUT = nc.dram_tensor("SOUT", [128, SW], F32, kind="ExternalOutput").ap()
    nchunk = T_run // U

    with tile.TileContext(nc) as tc, ExitStack() as ctx:
        const = ctx.enter_context(tc.tile_pool(name="const", bufs=1))
        state = ctx.enter_context(tc.tile_pool(name="state", bufs=1))
        xp = ctx.enter_context(tc.tile_pool(name="xp", bufs=2))
        xf = ctx.enter_context(tc.tile_pool(name="xf", bufs=2))
        op = ctx.enter_context(tc.tile_pool(name="op", bufs=2))
        wk = ctx.enter_context(tc.tile_pool(name="wk", bufs=2))
        psz = ctx.enter_context(tc.tile_pool(name="psz", bufs=1, space="PSUM"))
        pst = ctx.enter_context(tc.tile_pool(name="pst", bufs=2, space="PSUM"))
        psm = ctx.enter_context(tc.tile_pool(name="psm", bufs=1, space="PSUM"))

        wih = const.tile([128, 2, H4], F32)
        nc.sync.dma_start(wih[:, 0, :], WIHT[0:128, :])
        nc.sync.dma_start(wih[:, 1, :], WIHT[128:256, :])
        whh = const.tile([128, 2, H4], F32)
        nc.sync.dma_start(whh[:, 0, :], WHHT[0:128, :])
        nc.sync.dma_start(whh[:, 1, :], WHHT[128:256, :])
        biasr = const.tile([1, H4], F32)
        nc.sync.dma_start(biasr, BIAS)
        iota = const.tile([128, MEM], F32)
        nc.sync.dma_start(iota, IOTA)
        ident = const.tile([128, 128], F32)
        nc.sync.dma_start(ident, IDENT)
        ones1 = const.tile([1, 128], F32)
        nc.vector.memset(ones1, 1.0)

        st = state.tile([128, SW], F32)
        nc.sync.dma_start(st, SIN)
        Mpp = state.tile([128, 2, HID], F32)
        nc.vector.tensor_copy(out=Mpp[:, 0, :], in_=st[:, S_M:S_M + HID])

        c = st[:, S_C:S_C + HID]
        e_s = st[:, S_ES:S_ES + MEM]
        uP = st[:, S_UP:S_UP + MEM]
        rse = st[:, S_RSE:S_RSE + 1]
        ru = st[:, S_RU:S_RU + 1]

        def hT(k):
            return st[:, S_HT + k * 128:S_HT + (k + 1) * 128]

        def MT(k):
            return st[:, S_MT + k * 128:S_MT + (k + 1) * 128]

        def step(x_ap, o_ap, u):
            Mold = Mpp[:, u % 2, :]
            Mnew = Mpp[:, (u + 1) % 2, :]

            # (A) write weights from previous-step state
            negu = wk.tile([128, MEM], F32, tag="negu")
            nc.vector.tensor_scalar(negu, uP, -1.0, None, ALU.mult)
            m8 = wk.tile([128, 8], F32, tag="m8")
            nc.vector.max(m8, negu)
            i8 = wk.tile([128, 8], U32, tag="i8")
            nc.vector.max_index(i8, m8, negu)
            idxf = wk.tile([128, 1], F32, tag="idxf")
            nc.vector.tensor_copy(out=idxf, in_=i8[:, 0:1])
            onehot = wk.tile([128, MEM], F32, tag="onehot")
            nc.vector.tensor_scalar(onehot, iota, idxf, None, ALU.is_equal)
            grs = wk.tile([128, 1], F32, tag="grs")
            nc.vector.tensor_scalar(grs, rse, GATE, None, ALU.mult)
            gwr = wk.tile([128, MEM], F32, tag="gwr")
            nc.vector.tensor_scalar(gwr, e_s, grs, None, ALU.mult)
            w_w = wk.tile([128, MEM], F32, tag="w_w")
            nc.vector.scalar_tensor_tensor(w_w, onehot, 1.0 - GATE, gwr, ALU.mult, ALU.add)
            gru = wk.tile([128, 1], F32, tag="gru")
            nc.vector.tensor_scalar(gru, ru, GAMMA, None, ALU.mult)
            nc.vector.scalar_tensor_tensor(uP, uP, gru, w_w, ALU.mult, ALU.add)

            # (B) LSTM cell
            xT = wk.tile([128, 2, 128], F32, tag="xT")
            for k in range(2):
                tp = pst.tile([128, 128], F32, tag="tp")
                nc.tensor.transpose(tp, x_ap[:, k * 128:(k + 1) * 128], ident)
                nc.scalar.copy(xT[:, k, :], tp)

            zb = []
            for b_i in range(2):
                z = psz.tile([128, 512], F32, tag=f"z{b_i}")
                sl = slice(b_i * 512, (b_i + 1) * 512)
                nc.tensor.matmul(z, ones1, biasr[:, sl], start=True, stop=False)
                nc.tensor.matmul(z, xT[:, 0, :], wih[:, 0, sl], start=False, stop=False)
                nc.tensor.matmul(z, xT[:, 1, :], wih[:, 1, sl], start=False, stop=False)
                nc.tensor.matmul(z, hT(0), whh[:, 0, sl], start=False, stop=False)
                nc.tensor.matmul(z, hT(1), whh[:, 1, sl], start=False, stop=True)
                zb.append(z)
            z0, z1 = zb  # z0=[i,f], z1=[g,o]

            thif = wk.tile([128, 512], F32, tag="thif")
            nc.scalar.activation(thif, z0, AF.Tanh, scale=0.5)
            sif = wk.tile([128, 512], F32, tag="sif")
            nc.vector.tensor_scalar(sif, thif, 0.5, 0.5, ALU.mult, ALU.add)
            tg = wk.tile([128, 256], F32, tag="tg")
            nc.scalar.activation(tg, z1[:, 0:256], AF.Tanh)
            tho = wk.tile([128, 256], F32, tag="tho")
            nc.scalar.activation(tho, z1[:, 256:512], AF.Tanh, scale=0.5)
            so = wk.tile([128, 256], F32, tag="so")
            nc.vector.tensor_scalar(so, tho, 0.5, 0.5, ALU.mult, ALU.add)

            t1 = wk.tile([128, 256], F32, tag="t1")
            nc.vector.tensor_tensor(t1, sif[:, 256:512], c, ALU.mult)
            t2 = wk.tile([128, 256], F32, tag="t2")
            nc.vector.tensor_tensor(t2, sif[:, 0:256], tg, ALU.mult)
            nc.vector.tensor_tensor(c, t1, t2, ALU.add)
            tcn = wk.tile([128, 256], F32, tag="tcn")
            nc.scalar.activation(tcn, c, AF.Tanh)
            h = wk.tile([128, 256], F32, tag="h")
            nc.vector.tensor_tensor(h, so, tcn, ALU.mult)
            if OUT_FMT == 'u8':
                nc.vector.tensor_scalar(o_ap[:, 0:256], h, U8S, U8S + 0.5,
                                        ALU.mult, ALU.add)
            else:
                nc.vector.tensor_scalar(o_ap[:, 0:256], h, OSCALE, None, ALU.mult)

            nh = wk.tile([128, 1], F32, tag="nh")
            sq = wk.tile([128, 256], F32, tag="sq")
            nc.vector.scalar_tensor_tensor(sq, h, 1.0, h, ALU.mult, ALU.mult,
                                           accum_out=nh)
            rh = _emit_rsqrt(nc, wk, nh, 1, "rsH")

            for k in range(2):
                tp = pst.tile([128, 128], F32, tag="tp")
                nc.tensor.transpose(tp, h[:, k * 128:(k + 1) * 128], ident)
                nc.vector.tensor_copy(out=hT(k), in_=tp)

            # (C) read head against PRE-update M (reference ordering)
            ips = psm.tile([128, MEM], F32, tag="ips")
            nc.tensor.matmul(ips, hT(0), MT(0), start=True, stop=False)
            nc.tensor.matmul(ips, hT(1), MT(1), start=False, stop=True)
            sc = wk.tile([128, MEM], F32, tag="sc")
            nc.vector.tensor_scalar(sc, ips, rh, None, ALU.mult)
            mx = wk.tile([128, 1], F32, tag="mx")
            nc.vector.tensor_reduce(mx, sc, AX.X, ALU.max)
            bm = wk.tile([128, 1], F32, tag="bm")
            nc.vector.tensor_scalar(bm, mx, -1.0, None, ALU.mult)
            se = wk.tile([128, 1], F32, tag="se")
            nc.scalar.activation(e_s, sc, AF.Exp, bias=bm, scale=1.0, accum_out=se)
            nc.vector.reciprocal(rse, se)

            eT = wk.tile([128, MEM], F32, tag="eT")
            tp = pst.tile([128, 128], F32, tag="tp")
            nc.tensor.transpose(tp, e_s, ident)
            nc.vector.tensor_copy(out=eT, in_=tp)
            rps = psm.tile([128, 256], F32, tag="rps")
            nc.tensor.matmul(rps, eT, Mold, start=True, stop=True)
            if OUT_FMT == 'u8':
                rs127 = wk.tile([128, 1], F32, tag="rs127")
                nc.vector.tensor_scalar(rs127, rse, U8S, None, ALU.mult)
                nc.vector.tensor_scalar(o_ap[:, 256:512], rps, rs127, U8S + 0.5,
                                        ALU.mult, ALU.add)
            else:
                nc.vector.tensor_scalar(o_ap[:, 256:512], rps, rse, OSCALE,
                                        ALU.mult, ALU.mult)

            # (D) memory update (off the read critical path)
            dps = psm.tile([128, 256], F32, tag="dps")
            nc.tensor.matmul(dps, w_w, h, start=True, stop=True)
            MpD = wk.tile([128, 256], F32, tag="MpD")
            nc.vector.tensor_tensor(MpD, dps, Mold, ALU.add)
            nm = wk.tile([128, 1], F32, tag="nm")
            sqm = wk.tile([128, 256], F32, tag="sqm")
            nc.vector.scalar_tensor_tensor(sqm, MpD, 1.0, MpD, ALU.mult, ALU.mult,
                                           accum_out=nm)
            rm = _emit_rsqrt(nc, wk, nm, 1, "rsM")
            nc.vector.tensor_scalar(Mnew, MpD, rm, None, ALU.mult)
            for k in range(2):
                tp = pst.tile([128, 128], F32, tag="tp")
                nc.tensor.transpose(tp, Mnew[:, k * 128:(k + 1) * 128], ident)
                nc.vector.tensor_copy(out=MT(k), in_=tp)

            # (E) usage update
            nc.vector.scalar_tensor_tensor(uP, e_s, rse, uP, ALU.mult, ALU.add)
            nu = wk.tile([128, 1], F32, tag="nu")
            squ = wk.tile([128, MEM], F32, tag="squ")
            nc.vector.scalar_tensor_tensor(squ, uP, 1.0, uP, ALU.mult, ALU.mult,
                                           accum_out=nu)
            rb = _emit_rsqrt(nc, wk, nu, 1, "rsU")
            nc.vector.tensor_copy(out=ru, in_=rb)

        def chunk_body(x_dram_slice, out_dram_slice):
            xq = xp.tile([128, U, IN], I16)
            nc.sync.dma_start(xq, x_dram_slice)
            xt = xf.tile([128, U, IN], F32)
            nc.vector.tensor_copy(out=xt, in_=xq)
            ot = op.tile([128, U, 2 * HID], ODT)
            for u in range(U):
                step(xt[:, u, :], ot[:, u, :], u)
            nc.sync.dma_start(out_dram_slice, ot)

        if nchunk > 1:
            with tc.For_i(0, nchunk, 1, staggered_reset=True,
                          hint_engines=(mybir.EngineType.DVE,
                                        mybir.EngineType.PE,
                                        mybir.EngineType.Activation)) as ic:
                chunk_body(XQ[:, bass.ts(ic, U), :], OUT[:, bass.ts(ic, U), :])
        else:
            chunk_body(XQ[:, 0:U, :], OUT[:, 0:U, :])

        nc.vector.tensor_copy(out=st[:, S_M:S_M + HID], in_=Mpp[:, 0, :])
        nc.sync.dma_start(SOUT, st)

    nc.compile()
    return nc


def _get_engine():
    if "eng" in _CACHE:
        return _CACHE["eng"]
    import jax
    import jax.numpy as jnp
    from concourse.bass2jax import (_bass_exec_p, install_neuronx_cc_hook,
                                    partition_id_tensor)
    install_neuronx_cc_hook()
    nc = _build(T_SEG, U_UNROLL)
    in_names, out_names, out_avals, zero_specs = [], [], [], []
    for alloc in nc.m.functions[0].allocations:
        if not isinstance(alloc, mybir.MemoryLocationSet):
            continue
        name = alloc.memorylocations[0].name
        if alloc.kind == "ExternalInput":
            if name != "partition_id":
                in_names.append(name)
        elif alloc.kind == "ExternalOutput":
            out_names.append(name)
            shape = tuple(alloc.tensor_shape)
            dtype = mybir.dt.np(alloc.dtype)
            out_avals.append(jax.core.ShapedArray(shape, dtype))
            zero_specs.append((shape, dtype))
    n_params = len(in_names)
    has_pid = nc.partition_id_tensor is not None
    all_in_names = tuple(in_names + out_names
                         + (["partition_id"] if has_pid else []))

    def _body(*args):
        operands = list(args)
        if has_pid:
            operands.append(partition_id_tensor())
        return tuple(_bass_exec_p.bind(
            *operands,
            out_avals=tuple(out_avals),
            in_names=all_in_names,
            out_names=tuple(out_names),
            lowering_input_output_aliases=(),
            sim_require_finite=True,
            sim_require_nnan=True,
            nc=nc,
        ))

    donate = tuple(range(n_params, n_params + len(out_names)))
    jitted = jax.jit(_body, donate_argnums=donate, keep_unused=True)
    mk_zeros = jax.jit(
        lambda: tuple(jnp.zeros(s, d) for s, d in zero_specs))
    mk_state0 = jax.jit(lambda: jnp.zeros((128, SW), np.float32))
    eng = {"nc": nc, "jitted": jitted, "mk_zeros": mk_zeros,
           "mk_state0": mk_state0, "in_names": in_names,
           "out_names": out_names, "jax": jax}
    _CACHE["eng"] = eng
    return eng


def kernel(X, W_ih, W_hh, b_ih, b_hh):
    eng = _get_engine()
    jax = eng["jax"]
    dev = jax.devices()[0]

    X = np.asarray(X, np.float32)
    amax = float(np.abs(X).max())
    xs = amax / 32767.0 if amax > 0 else 1.0
    XQ = np.rint(X * (1.0 / xs)).astype(np.int16)

    consts = {
        "WIHT": np.ascontiguousarray(np.asarray(W_ih, np.float32).T) * np.float32(xs),
        "WHHT": np.ascontiguousarray(np.asarray(W_hh, np.float32).T),
        "BIAS": np.ascontiguousarray(
            (np.asarray(b_ih, np.float32) + np.asarray(b_hh, np.float32)
             ).reshape(1, H4)),
        "IOTA": np.tile(np.arange(MEM, dtype=np.float32), (128, 1)),
        "IDENT": np.eye(128, dtype=np.float32),
    }
    dev_consts = {k: jax.device_put(v, dev) for k, v in consts.items()}

    # enqueue per-segment H2D of X up front (async; tunnel streams in order)
    xsegs = [jax.device_put(
        np.ascontiguousarray(XQ[:, s * T_SEG:(s + 1) * T_SEG, :]), dev)
        for s in range(N_SEG)]

    state = eng["mk_state0"]()
    outs = []
    for s in range(N_SEG):
        zeros = eng["mk_zeros"]()
        in_map = dict(dev_consts)
        in_map["XQ"] = xsegs[s]
        in_map["SIN"] = state
        args = [in_map[n] for n in eng["in_names"]]
        res = eng["jitted"](*args, *zeros)
        res_map = dict(zip(eng["out_names"], res))
        o = res_map["OUT"]
        o.copy_to_host_async()
        outs.append(o)
        state = res_map["SOUT"]

    full = np.empty((B, T, 2 * HID), np.float32)
    for s, o in enumerate(outs):
        dst = full[:, s * T_SEG:(s + 1) * T_SEG, :]
        if OUT_FMT == 'u8':
            np.multiply(np.asarray(o), np.float32(1.0 / U8S), out=dst)
            dst -= np.float32(1.0)
        else:
            np.multiply(np.asarray(o), np.float32(1.0 / OSCALE), out=dst)
    return full
